# revision 22
# baseline (speedup 1.0000x reference)
"""Trainium2 Bass kernel for nn_CurvStdDist (retrieval_knn) — v4.

Reference computation (per batch b, per cloud):
  x: (n,3) points, nrm: (n,3) unit normals, k=16
  idx   = 16 nearest neighbors of each point (excluding self, by squared L2)
  v     = x[idx] - x[:,None]; vhat = v / clip(||v||, 1e-12)
  kappa = mean_k |vhat . nrm|                      (n,)
  std   = std(kappa[idx], ddof=1)                  (n,)
Final: dist = mean_b ||ori_std[b] - adv_std[b] + 1e-6||_2

Sharding: 8 cores = 4 batches x 2 clouds (ori/adv); each core runs the
full n=4096 KNN pipeline for one (batch, cloud); host combines the 8
std vectors into the scalar.

v4.2 = v3 with four changes (measured 824us -> 749us single-shot NTFF;
rel err 6.5e-3 -> 5.4e-3):
  - fp16 matmuls (1 col/cyc vs ~4 for fp32r/fp32): rhs carries the
    per-column |x_j|^2 split hi/lo so it keeps ~fp32 precision; the
    per-row -|x_i|^2 term rides lhsT in fp16 — its quantization shifts
    whole rows only, never the within-row ordering.  K=6 contraction.
    Diagonal self-exclusion uses -60000 (fits fp16).
  - top-k scans on 1024-wide double psum banks ([128,1024] max8 +
    max_index): 4 scans/tile instead of 8, 32 merge candidates.
    Per-chunk top-8 insufficiency grows to ~0.3% of rows, each of which
    only swaps in a ~17th-nearest neighbor.
  - variable-width gather calls (7x4 tiles + 2x2 tiles): the two small
    trailing calls halve the post-scan serialized tail (the Q7 ap_gather
    is the whole-kernel bottleneck at ~27ns/idx + ~11ns/extra-elem, so
    the last xyz gather's latency is directly exposed before phase B).
  - neighbor-coordinate fetches from an fp16 d=4 table (8B/idx) instead
    of fp32 d=3 (12B/idx): ~22%% off the xyz-gather Q7 time.  Only the
    NEIGHBOR side of v = x_j - x_i quantizes (own coords stay fp32 via
    xi_all); |v| ~ 0.4 typical vs ~1e-3 coord error -> ~+2e-4 on the
    final scalar (measured).  An ACT copy upcasts the gathered fp16
    block before the kappa chain.

Everything else is v3: chunked top-k with packed-key merge
(key = 4096*(2047+round(v*256)) + j, exact fp32 integers), ap_gather
for all neighbor fetches against replicated SBUF tables, kappa on a
wide DVE chain, phase-B neighbor-kappa std with ddof=1.

Perf notes for future sessions (all HW-measured here):
  - gpsimd ap_gather ucode is command-bound: ~130 cyc per 4-idx group
    (reset_reads+reset_write), giving 27ns/idx at d=1 and 49ns/idx at
    d=3, per Q7 core (8 cores run per-core index lists in parallel).
    Q7 total = 8 xyz calls (~400us) + phase-B kappa calls (~220us) and
    is THE kernel floor; DVE scans are ~370us, ACT ~215us, PE ~150us.
  - dma_gather (SWDGE descriptor gather) measures ~8ns/idx desc-gen,
    single-threaded on Q7 — strictly worse than ap_gather here; also
    crashes with single_packet=True above 1024 idxs (64 descs/engine
    ring limit).  bf16 gives NO DVE speedup for MAX8/FIND_INDEX8 (1x
    only).  The axon-tunnel wall-clock drifts minute-to-minute, so only
    the NTFF device exec time is a trustworthy metric.
  - one profiled cold run of this version produced a corrupted result
    (rel 3.3) that never reproduced: 20/20 stress runs + all normal
    executions are exact.  Suspect NTFF-capture timing perturbation;
    re-verify with /tmp/stress.py-style loops if touching DMA deps.
"""

import numpy as np

N = 4096          # points per cloud
P = 128           # partitions
T = N // P        # 32 row tiles
K = 16            # neighbors
CHUNK = 1024      # top-k chunk width (2 psum banks)
NCHUNK = N // CHUNK
DIAG_NEG = -60000.0  # added on the diagonal (self distance); fits fp16
FILL_NEG = -3.0e38   # match_replace fill
SQ = 256.0           # d2 quantization scale for the merge keys
BRND = 3.0 * 2.0**22  # fp32 round-to-int bias
B4096 = BRND * 4096.0

_PROG_CACHE = {}


def _build_program(stage="full", reps=1):
    """Build + compile the single-core Bass program (shared by all 8 cores).

    stage: "mm" | "topk" | "idx" | "gather" | "full" — debug prefixes of
    the pipeline; anything but "full" writes intermediate checksums instead.
    reps: repeat the whole pipeline (timing harness: marginal wall per rep).
    """
    import concourse.bacc as bacc
    import concourse.bass as bass
    import concourse.mybir as mybir
    import concourse.tile as tile
    from concourse.tile_rust import add_dep_helper

    dt = mybir.dt
    AF = mybir.ActivationFunctionType
    Alu = mybir.AluOpType

    nc = bacc.Bacc("TRN2", target_bir_lowering=False, debug=False)

    lhsT6 = nc.dram_tensor("lhsT6", [6, N], dt.float16, kind="ExternalInput")
    rhs6 = nc.dram_tensor("rhs6", [6, N], dt.float16, kind="ExternalInput")
    xyz = nc.dram_tensor("xyz", [N, 3], dt.float32, kind="ExternalInput")
    # fp16 copy of the coordinates, padded to 4 components, for the
    # neighbor-fetch table: ap_gather cost is ~11ns per 4B word per idx,
    # so d=4 fp16 (8B) beats d=3 fp32 (12B) by ~22%% on the Q7 floor.
    # Only neighbor coords quantize; own-point coords stay exact fp32.
    xyz16 = nc.dram_tensor("xyz16", [N, 4], dt.float16, kind="ExternalInput")
    nrm = nc.dram_tensor("nrm", [N, 3], dt.float32, kind="ExternalInput")
    eye = nc.dram_tensor("eye", [P, P], dt.float16, kind="ExternalInput")
    # DIAG_NEG*I at columns 384:512 of a zero [P, 896]; slicing
    # [384-off : 896-off] yields a [P, 512] half-chunk row with the negative
    # diagonal at columns off:off+P
    negpad = nc.dram_tensor("negpad", [P, 896], dt.float16, kind="ExternalInput")
    # per-column chunk offset (c//8)*1024 to globalize chunk-local indices
    choff = nc.dram_tensor("choff", [P, 32], dt.float32, kind="ExternalInput")
    # quadrant mask for the wrapped-list build: 1 where p-half XOR f-half
    qmask = nc.dram_tensor("qmask", [P, 64], dt.float32, kind="ExternalInput")
    kap_d = nc.dram_tensor("kappa", [N, 1], dt.float32, kind="ExternalOutput")
    std_d = nc.dram_tensor("std", [N, 1], dt.float32, kind="ExternalOutput")

    def bcast_mid(ap, k):
        # [P, (1,) c] -> [P, k, c] with a stride-0 middle dim
        return bass.AP(ap.tensor, ap.offset, [ap.ap[0], [0, k], ap.ap[-1]])

    def dram_bcast(t_handle, n_elem):
        # DRAM tensor broadcast to all 128 partitions: [[0,128],[1,n]]
        ap = t_handle.ap()
        return bass.AP(ap.tensor, 0, [[0, P], [1, n_elem]])

    with tile.TileContext(nc) as tc:
        with (
            tc.tile_pool(name="const", bufs=1) as constp,
            tc.tile_pool(name="srow", bufs=4) as sp,
            tc.tile_pool(name="psum", bufs=4, space="PSUM") as pp,
            tc.tile_pool(name="small", bufs=4) as smp,
            tc.tile_pool(name="call", bufs=2) as callp,
            tc.tile_pool(name="rep", bufs=2) as repp,
            tc.tile_pool(name="idxp", bufs=1) as idxp,
        ):
            lh = constp.tile_from(lhsT6.ap())
            rh = constp.tile_from(rhs6.ap())
            ey = constp.tile_from(eye.ap())
            npd = constp.tile_from(negpad.ap())
            co = constp.tile_from(choff.ap())
            qm = constp.tile_from(qmask.ap())
            # replicated coordinate table: every partition holds all of
            # xyz16 (fp16, 4 components per point)
            xyztab = constp.tile([P, N * 4], dt.float16)
            nc.sync.dma_start(xyztab[:], dram_bcast(xyz16, N * 4))
            # wrapped int16 index lists for all 16 ap_gather calls
            L_all = idxp.tile([P, (T // 4) * 64], dt.int16)
            kaptab = idxp.tile([P, N], dt.float32)
            # all tiles' own coords/normals in one DMA: [p, t, c] <- row t*P+p
            xi_all = constp.tile([P, T, 3], dt.float32)
            nc.sync.dma_start(
                xi_all[:], xyz.ap().rearrange("(t p) c -> p t c", p=P)
            )
            ni_all = constp.tile([P, T, 3], dt.float32)
            nc.sync.dma_start(
                ni_all[:], nrm.ap().rearrange("(t p) c -> p t c", p=P)
            )

            # call schedule: 7 calls of 4 tiles, then 2 calls of 2 tiles —
            # the smaller final gathers shorten the post-scan tail.
            call_plan = [(4 * c, 4) for c in range(7)] + [(28, 2), (30, 2)]
            call_w0 = {cs: w for cs, w in call_plan}
            for _rep in range(reps):
                # ---------------- phase A: knn + kappa ----------------
                jf_call = None
                cur_cs, cur_w, cur_col = 0, 4, 0
                kap_stores = []
                for t in range(T):
                    cand = smp.tile([P, 32], dt.float32, tag="cand")
                    candi = smp.tile([P, 32], dt.uint32, tag="candi")
                    if t in call_w0:
                        if jf_call is not None:
                            cur_col += 16 * cur_w
                        cur_cs, cur_w = t, call_w0[t]
                        jf_call = callp.tile(
                            [P, 16 * cur_w], dt.float32, tag=f"jf{cur_w}",
                            name=f"jf{cur_w}_{t}",
                        )
                    cd, off = (t * P) // CHUNK, (t * P) % CHUNK
                    for c in range(NCHUNK):
                        ps = pp.tile([P, CHUNK], dt.float32, tag="ps")
                        for h in range(2):
                            b = 2 * c + h
                            hs = ps[:, h * 512 : (h + 1) * 512]
                            diag_here = c == cd and (off // 512) == h
                            hoff = off % 512
                            nc.tensor.matmul(
                                out=hs,
                                lhsT=lh[:, t * P : (t + 1) * P],
                                rhs=rh[:, b * 512 : (b + 1) * 512],
                                start=True,
                                stop=not diag_here,
                            )
                            if diag_here:
                                nc.tensor.matmul(
                                    out=hs,
                                    lhsT=ey[:],
                                    rhs=npd[:, 384 - hoff : 896 - hoff],
                                    start=False,
                                    stop=True,
                                )
                        Sbt = sp.tile([P, CHUNK], dt.float32, tag="Sb")
                        Sb = Sbt[:]
                        nc.scalar.copy(Sb, ps[:])
                        # per-chunk top-8 values + chunk-local indices
                        nc.vector.max(cand[:, c * 8 : c * 8 + 8], Sb)
                        nc.vector.max_index(
                            candi[:, c * 8 : c * 8 + 8],
                            cand[:, c * 8 : c * 8 + 8],
                            Sb,
                        )

                    if stage == "mm":
                        # per-chunk top-8 already captures the row max
                        chk = smp.tile([P, 1], dt.float32, tag="chk")
                        nc.vector.tensor_reduce(
                            chk[:], cand[:], axis=mybir.AxisListType.X, op=Alu.max
                        )
                        nc.sync.dma_start(std_d.ap()[t * P : (t + 1) * P, :], chk[:])
                        continue

                    # merge by packed keys: key = 4096*(2047 + round(v*256)) + j
                    candif = smp.tile([P, 32], dt.float32, tag="candif")
                    nc.scalar.copy(candif[:], candi[:])
                    candg = smp.tile([P, 32], dt.float32, tag="candg")
                    nc.vector.tensor_tensor(
                        out=candg[:], in0=candif[:], in1=co[:], op=Alu.add
                    )
                    q = smp.tile([P, 32], dt.float32, tag="q")
                    nc.scalar.activation(
                        q[:], cand[:], AF.Copy, bias=2047.0 + BRND, scale=SQ
                    )
                    t1 = smp.tile([P, 32], dt.float32, tag="t1")
                    nc.scalar.activation(
                        t1[:], q[:], AF.Copy, bias=-B4096, scale=4096.0
                    )
                    key = smp.tile([P, 32], dt.float32, tag="key")
                    nc.vector.tensor_tensor(
                        out=key[:], in0=t1[:], in1=candg[:], op=Alu.add
                    )
                    k16 = smp.tile([P, 16], dt.float32, tag="k16")
                    key_mr = smp.tile([P, 32], dt.float32, tag="key_mr")
                    nc.vector.max(k16[:, 0:8], key[:])
                    nc.vector.match_replace(key_mr[:], k16[:, 0:8], key[:], FILL_NEG)
                    nc.vector.max(k16[:, 8:16], key_mr[:])

                    if stage == "topk":
                        chk = smp.tile([P, 1], dt.float32, tag="chk")
                        nc.vector.tensor_reduce(
                            chk[:], k16[:], axis=mybir.AxisListType.X, op=Alu.add
                        )
                        nc.sync.dma_start(std_d.ap()[t * P : (t + 1) * P, :], chk[:])
                        continue

                    # unpack j = k16 mod 4096 (exact fp32 integer arithmetic)
                    u1 = smp.tile([P, 16], dt.float32, tag="u1")
                    nc.scalar.activation(
                        u1[:], k16[:], AF.Copy,
                        bias=-0.4998779296875, scale=1.0 / 4096.0,
                    )
                    u2a = smp.tile([P, 16], dt.float32, tag="u2a")
                    nc.scalar.activation(u2a[:], u1[:], AF.Copy, bias=BRND)
                    u2 = smp.tile([P, 16], dt.float32, tag="u2")
                    nc.scalar.activation(u2[:], u2a[:], AF.Copy, bias=-BRND)
                    tl = t - cur_cs
                    jf = jf_call[:, tl * 16 : tl * 16 + 16]
                    nc.vector.scalar_tensor_tensor(
                        out=jf, in0=u2[:], scalar=-4096.0, in1=k16[:],
                        op0=Alu.mult, op1=Alu.add,
                    )

                    if stage == "idx":
                        chk = smp.tile([P, 1], dt.float32, tag="chk")
                        nc.vector.tensor_reduce(
                            chk[:], jf, axis=mybir.AxisListType.X, op=Alu.add
                        )
                        nc.sync.dma_start(std_d.ap()[t * P : (t + 1) * P, :], chk[:])
                        continue

                    if t != cur_cs + cur_w - 1:
                        continue

                    # ---- per W-tile call: list build + gather + kappa ----
                    W = cur_w
                    WC = 16 * W  # list columns
                    # wrapped list Lf[32A+ah*16+k, mh*16+b] =
                    #   jf_call[32A+ah*16+b, mh*16+k] = o32[32A+mh*16+k, ah*16+b]
                    # via 32x32 stream transpose + 16-lane shuffle + quad copies
                    o32 = smp.tile([P, WC], dt.float32, tag=f"o32w{W}",
                                   name=f"o32w{W}_{t}")
                    nc.vector.transpose(o32[:], jf_call[:])
                    sh = smp.tile([P, WC], dt.float32, tag=f"shw{W}",
                                  name=f"shw{W}_{t}")
                    nc.vector.stream_shuffle(
                        sh[:], o32[:], mask=[(i + 16) % 32 for i in range(32)]
                    )
                    # diag quads from o32; off-diag quads (partition-half XOR
                    # free-half) overwritten from the 16-lane-shifted copy,
                    # whose free-halves swap: use a free-half-swapped view of
                    # sh so one predicated copy works
                    Lf = smp.tile([P, WC], dt.float32, tag=f"Lfw{W}",
                                  name=f"Lfw{W}_{t}")
                    nc.scalar.copy(Lf[:], o32[:])
                    # Lf = o32 + qm * (sh_freehalfswap - o32)   (exact: ints)
                    sh_ap = sh[:]
                    sh_sw = bass.AP(
                        sh_ap.tensor, sh_ap.offset + 16,
                        [sh_ap.ap[0], [32, WC // 32], [-16, 2], [1, 16]],
                    )
                    dq = smp.tile([P, WC], dt.float32, tag=f"dqw{W}",
                                  name=f"dqw{W}_{t}")
                    dq3 = dq[:].rearrange("p (f h b) -> p f h b", h=2, b=16)
                    nc.vector.tensor_tensor(
                        out=dq3, in0=sh_sw,
                        in1=Lf[:].rearrange("p (f h b) -> p f h b", h=2, b=16),
                        op=Alu.subtract,
                    )
                    nc.vector.tensor_tensor(
                        out=dq[:], in0=dq[:], in1=qm[:, 0:WC], op=Alu.mult
                    )
                    nc.vector.tensor_tensor(
                        out=Lf[:], in0=Lf[:], in1=dq[:], op=Alu.add
                    )
                    L16 = L_all[:, cur_col : cur_col + WC]
                    nc.scalar.copy(L16, Lf[:])

                    # one d=3 ap_gather: rep3[p in core a, (mh*16+b)*16+k] =
                    #   xyz[idx of point((cs+mh)*128 + a*16+b, k)]
                    rep3 = callp.tile([P, W * 256 * 4], dt.float16,
                                      tag=f"rep3w{W}", name=f"rep3w{W}_{t}")
                    nc.gpsimd.ap_gather(
                        out_ap=rep3[:],
                        in_ap=xyztab[:],
                        idxs_ap=L16,
                        channels=P,
                        num_elems=N,
                        d=4,
                        num_idxs=W * 256,
                    )
                    # compact to per-point layout: nnc16[a*16+b, mh*64+kz] =
                    #   rep3[a*16, mh*1024 + b*64 + kz]   (64 fp16 per point)
                    nnc16 = callp.tile([P, W * K * 4], dt.float16,
                                       tag=f"nnc16w{W}", name=f"nnc16w{W}_{t}")
                    r3ap = rep3[:]
                    for mh in range(W):
                        src = bass.AP(
                            r3ap.tensor,
                            r3ap.offset + mh * 1024,
                            [[r3ap.ap[0][0] * 16, 8], [64, 16], [1, 64]],
                        )
                        nc.sync.dma_start(
                            nnc16[:, mh * 64 : (mh + 1) * 64], src
                        )
                    nnc = callp.tile([P, W * K * 4], dt.float32,
                                     tag=f"nncw{W}", name=f"nncw{W}_{t}")
                    nc.scalar.copy(nnc[:], nnc16[:])

                    if stage == "gather":
                        for mh in range(W):
                            tt = cur_cs + mh
                            chk = smp.tile([P, 1], dt.float32, tag="chk")
                            nc.vector.tensor_reduce(
                                chk[:], nnc[:, mh * 64 : (mh + 1) * 64],
                                axis=mybir.AxisListType.X, op=Alu.add,
                            )
                            nc.sync.dma_start(
                                std_d.ap()[tt * P : (tt + 1) * P, :], chk[:]
                            )
                        continue

                    # kappa for all W tiles of the call in one wide chain
                    # (nnc holds 4 components per neighbor; view picks 0:3)
                    nnc_ap = nnc[:]
                    nn4 = bass.AP(
                        nnc_ap.tensor, nnc_ap.offset,
                        [nnc_ap.ap[0], [64, W], [4, K], [1, 3]],
                    )
                    xi4 = xi_all[:, cur_cs : cur_cs + W, :]
                    ni4 = ni_all[:, cur_cs : cur_cs + W, :]

                    def bcast_k4(ap):
                        # [P, W, 3] -> [P, W, K, 3] with a stride-0 K dim
                        return bass.AP(
                            ap.tensor, ap.offset,
                            [ap.ap[0], ap.ap[1], [0, K], ap.ap[2]],
                        )

                    v = smp.tile([P, W * K * 3], dt.float32, tag=f"vw{W}",
                                 name=f"vw{W}_{t}")
                    v4 = v[:].rearrange("p (t k c) -> p t k c", k=K, c=3)
                    nc.vector.tensor_tensor(
                        out=v4, in0=nn4, in1=bcast_k4(xi4), op=Alu.subtract
                    )
                    vn = smp.tile([P, W * K * 3], dt.float32, tag=f"vnw{W}",
                                  name=f"vnw{W}_{t}")
                    vn4 = vn[:].rearrange("p (t k c) -> p t k c", k=K, c=3)
                    nc.vector.tensor_tensor(
                        out=vn4, in0=v4, in1=bcast_k4(ni4), op=Alu.mult
                    )
                    dot = smp.tile([P, W * K], dt.float32, tag=f"dotw{W}",
                                   name=f"dotw{W}_{t}")
                    nc.vector.tensor_reduce(
                        dot[:].rearrange("p (t k) -> p t k", k=K), vn4,
                        axis=mybir.AxisListType.X, op=Alu.add,
                    )
                    v2 = smp.tile([P, W * K * 3], dt.float32, tag=f"v2w{W}",
                                  name=f"v2w{W}_{t}")
                    v24 = v2[:].rearrange("p (t k c) -> p t k c", k=K, c=3)
                    nc.vector.tensor_tensor(out=v24, in0=v4, in1=v4, op=Alu.mult)
                    n2 = smp.tile([P, W * K], dt.float32, tag=f"n2w{W}",
                                  name=f"n2w{W}_{t}")
                    nc.vector.tensor_reduce(
                        n2[:].rearrange("p (t k) -> p t k", k=K), v24,
                        axis=mybir.AxisListType.X, op=Alu.add,
                    )
                    # clip ||v||^2 at 1e-24 (reference clips ||v|| at 1e-12)
                    nc.vector.tensor_scalar_max(n2[:], n2[:], 1e-24)
                    ri = smp.tile([P, W * K], dt.float32, tag=f"riw{W}",
                                  name=f"riw{W}_{t}")
                    nc.vector.reciprocal(ri[:], n2[:])
                    rs = smp.tile([P, W * K], dt.float32, tag=f"rsw{W}",
                                  name=f"rsw{W}_{t}")
                    nc.scalar.activation(rs[:], ri[:], AF.Sqrt)
                    sc = smp.tile([P, W * K], dt.float32, tag=f"scw{W}",
                                  name=f"scw{W}_{t}")
                    nc.vector.tensor_tensor(
                        out=sc[:], in0=dot[:], in1=rs[:], op=Alu.mult
                    )
                    kap = smp.tile([P, W], dt.float32, tag=f"kapw{W}",
                                   name=f"kapw{W}_{t}")
                    nc.vector.tensor_reduce(
                        kap[:].rearrange("p (t o) -> p t o", o=1),
                        sc[:].rearrange("p (t k) -> p t k", k=K),
                        axis=mybir.AxisListType.X,
                        op=Alu.add,
                        apply_absolute_value=True,
                    )  # = 16 * kappa
                    for mh in range(W):
                        tt = cur_cs + mh
                        kst = nc.sync.dma_start(
                            kap_d.ap()[tt * P : (tt + 1) * P, :],
                            kap[:, mh : mh + 1],
                        )
                        kap_stores.append(kst)

                # ---------------- phase B: neighbor-kappa std ----------------
                ncall = 4 if stage == "full" else 0
                if ncall:
                    # kaptab broadcast in 8 chunks, each only waiting on its
                    # own 4 tiles' kappa stores -> overlaps phase A
                    kap_ap = kap_d.ap()
                    for qc in range(8):
                        seg = bass.AP(
                            kap_ap.tensor, qc * 512, [[0, P], [1, 512]]
                        )
                        kb = nc.sync.dma_start(
                            kaptab[:, qc * 512 : (qc + 1) * 512], seg
                        )
                        for kst in kap_stores[qc * 4 : (qc + 1) * 4]:
                            add_dep_helper(kb.ins, kst.ins, True,
                                           "kaptab chunk after kappa stores")
                for call in range(ncall):
                    rep1 = repp.tile([P, 2048], dt.float32, tag="rep1")
                    nc.gpsimd.ap_gather(
                        out_ap=rep1[:],
                        in_ap=kaptab[:],
                        idxs_ap=L_all[:, call * 128 : (call + 1) * 128],
                        channels=P,
                        num_elems=N,
                        d=1,
                        num_idxs=2048,
                    )
                    nnk = smp.tile([P, 8 * K], dt.float32, tag="nnk")
                    r1ap = rep1[:]
                    for mh in range(8):
                        src = bass.AP(
                            r1ap.tensor,
                            r1ap.offset + mh * 256,
                            [[r1ap.ap[0][0] * 16, 8], [16, 16], [1, 16]],
                        )
                        eng = nc.sync if mh % 2 == 0 else nc.scalar
                        eng.dma_start(nnk[:, mh * 16 : (mh + 1) * 16], src)
                    # wide std math for all 8 tiles of the call:
                    # ss = sum k^2 - 16*mean^2  (safe: ~6e-6 rel cancellation)
                    nnk3 = nnk[:].rearrange("p (t k) -> p t k", k=K)
                    sm = smp.tile([P, 8], dt.float32, tag="sm")
                    nc.vector.tensor_reduce(
                        sm[:], nnk3, axis=mybir.AxisListType.X, op=Alu.add
                    )
                    mn = smp.tile([P, 8], dt.float32, tag="mn")
                    nc.vector.tensor_scalar_mul(mn[:], sm[:], 1.0 / K)
                    sq = smp.tile([P, 8 * K], dt.float32, tag="sq")
                    nc.vector.tensor_tensor(
                        out=sq[:], in0=nnk[:], in1=nnk[:], op=Alu.mult
                    )
                    r2 = smp.tile([P, 8], dt.float32, tag="r2")
                    nc.vector.tensor_reduce(
                        r2[:], sq[:].rearrange("p (t k) -> p t k", k=K),
                        axis=mybir.AxisListType.X, op=Alu.add,
                    )
                    m2 = smp.tile([P, 8], dt.float32, tag="m2")
                    nc.vector.tensor_tensor(
                        out=m2[:], in0=mn[:], in1=mn[:], op=Alu.mult
                    )
                    ss = smp.tile([P, 8], dt.float32, tag="ss")
                    nc.vector.scalar_tensor_tensor(
                        out=ss[:], in0=m2[:], scalar=-float(K), op0=Alu.mult,
                        in1=r2[:], op1=Alu.add,
                    )
                    stdt = smp.tile([P, 8], dt.float32, tag="stdt")
                    # std = sqrt(ss/(K-1))/K  (kappa was stored scaled by K)
                    nc.scalar.activation(
                        stdt[:], ss[:], AF.Sqrt, scale=1.0 / ((K - 1) * K * K)
                    )
                    for mh in range(8):
                        tt = 8 * call + mh
                        nc.sync.dma_start(
                            std_d.ap()[tt * P : (tt + 1) * P, :],
                            stdt[:, mh : mh + 1],
                        )

    nc.compile()
    return nc


def get_program():
    if "nc" not in _PROG_CACHE:
        _PROG_CACHE["nc"] = _build_program()
    return _PROG_CACHE["nc"]


def make_in_map(x3n: np.ndarray, nrm3n: np.ndarray) -> dict:
    """Per-core inputs. x3n, nrm3n: (3, N) float32."""
    x = np.ascontiguousarray(x3n, dtype=np.float32)          # (3, N)
    xyz = np.ascontiguousarray(x.T)                          # (N, 3)
    nrm = np.ascontiguousarray(np.asarray(nrm3n, np.float32).T)
    sq = (x * x).sum(axis=0, dtype=np.float32)               # (N,)
    sq_hi = sq.astype(np.float16).astype(np.float32)
    sq_lo = sq - sq_hi
    ones = np.ones((N,), np.float32)
    rhs6 = np.ascontiguousarray(
        np.stack([x[0], x[1], x[2], ones, sq_hi, sq_lo]).astype(np.float16)
    )
    lhsT6 = np.ascontiguousarray(
        np.stack([2 * x[0], 2 * x[1], 2 * x[2], -sq, -ones, -ones]).astype(
            np.float16
        )
    )
    eye = np.eye(P, dtype=np.float16)
    negpad = np.zeros((P, 896), np.float16)
    negpad[:, 384:512] = np.float16(DIAG_NEG) * np.eye(P, dtype=np.float16)
    choff = np.broadcast_to(
        ((np.arange(32) // 8) * CHUNK).astype(np.float32), (P, 32)
    ).copy()
    pq = (np.arange(P)[:, None] % 32 >= 16)
    fq = (np.arange(64)[None, :] % 32 >= 16)
    qmask = (pq ^ fq).astype(np.float32)
    xyz16 = np.zeros((N, 4), np.float16)
    xyz16[:, 0:3] = xyz.astype(np.float16)
    return {
        "lhsT6": lhsT6,
        "rhs6": rhs6,
        "xyz": xyz,
        "xyz16": xyz16,
        "nrm": nrm,
        "eye": eye,
        "negpad": negpad,
        "choff": choff,
        "qmask": qmask,
    }


def combine(std_vecs: list) -> np.ndarray:
    """std_vecs: 8 arrays (N,) — cores 0-3 ori batches, 4-7 adv batches."""
    dists = []
    for b in range(4):
        diff = (
            std_vecs[b].astype(np.float64)
            - std_vecs[4 + b].astype(np.float64)
            + 1e-6
        )
        dists.append(np.sqrt((diff * diff).sum()))
    return np.asarray(np.mean(dists), dtype=np.float32)


def kernel(ori_data, adv_data, ori_normal):
    from concourse.bass_utils import run_bass_kernel_spmd

    nc = get_program()
    in_maps = []
    for cloud in (ori_data, adv_data):
        for b in range(4):
            in_maps.append(make_in_map(cloud[b], ori_normal[b]))
    res = run_bass_kernel_spmd(nc, in_maps, core_ids=list(range(8)))
    std_vecs = [r["std"][:, 0] for r in res.results]
    return combine(std_vecs)


# revision 24
# speedup vs baseline: 2.5381x; 2.5381x over previous
"""Trainium2 Bass kernel for nn_CurvStdDist (retrieval_knn) — v4.

Reference computation (per batch b, per cloud):
  x: (n,3) points, nrm: (n,3) unit normals, k=16
  idx   = 16 nearest neighbors of each point (excluding self, by squared L2)
  v     = x[idx] - x[:,None]; vhat = v / clip(||v||, 1e-12)
  kappa = mean_k |vhat . nrm|                      (n,)
  std   = std(kappa[idx], ddof=1)                  (n,)
Final: dist = mean_b ||ori_std[b] - adv_std[b] + 1e-6||_2

Sharding: 8 cores = 4 batches x 2 clouds (ori/adv); each core runs the
full n=4096 KNN pipeline for one (batch, cloud); host combines the 8
std vectors into the scalar.

v4.3 = v3 with four changes (measured 824us -> 735us single-shot NTFF;
rel err 6.5e-3 -> 5.4e-3):
  - fp16 matmuls (1 col/cyc vs ~4 for fp32r/fp32): rhs carries the
    per-column |x_j|^2 split hi/lo so it keeps ~fp32 precision; the
    per-row -|x_i|^2 term rides lhsT in fp16 — its quantization shifts
    whole rows only, never the within-row ordering.  K=6 contraction.
    Diagonal self-exclusion uses -60000 (fits fp16).
  - top-k scans on 1024-wide double psum banks ([128,1024] max8 +
    max_index): 4 scans/tile instead of 8, 32 merge candidates.
    Per-chunk top-8 insufficiency grows to ~0.3% of rows, each of which
    only swaps in a ~17th-nearest neighbor.
  - variable-width gather calls (6x4 tiles + 4x2 tiles): the small
    trailing calls halve the post-scan serialized tail (the Q7 ap_gather
    is the whole-kernel bottleneck at ~27ns/idx + ~11ns/extra-elem, so
    the last xyz gather's latency is directly exposed before phase B).
  - neighbor-coordinate fetches from an fp16 d=4 table (8B/idx) instead
    of fp32 d=3 (12B/idx): ~22%% off the xyz-gather Q7 time.  Only the
    NEIGHBOR side of v = x_j - x_i quantizes (own coords stay fp32 via
    xi_all); |v| ~ 0.4 typical vs ~1e-3 coord error -> ~+2e-4 on the
    final scalar (measured).  An ACT copy upcasts the gathered fp16
    block before the kappa chain.

Everything else is v3: chunked top-k with packed-key merge
(key = 4096*(2047+round(v*256)) + j, exact fp32 integers), ap_gather
for all neighbor fetches against replicated SBUF tables, kappa on a
wide DVE chain, phase-B neighbor-kappa std with ddof=1.

Perf notes for future sessions (all HW-measured here):
  - gpsimd ap_gather ucode is command-bound: ~130 cyc per 4-idx group
    (reset_reads+reset_write), giving 27ns/idx at d=1 and 49ns/idx at
    d=3, per Q7 core (8 cores run per-core index lists in parallel).
    Q7 total = 8 xyz calls (~400us) + phase-B kappa calls (~220us) and
    is THE kernel floor; DVE scans are ~370us, ACT ~215us, PE ~150us.
  - dma_gather (SWDGE descriptor gather) measures ~8ns/idx desc-gen,
    single-threaded on Q7 — strictly worse than ap_gather here; also
    crashes with single_packet=True above 1024 idxs (64 descs/engine
    ring limit).  bf16 gives NO DVE speedup for MAX8/FIND_INDEX8 (1x
    only).  The axon-tunnel wall-clock drifts minute-to-minute, so only
    the NTFF device exec time is a trustworthy metric.
  - one profiled cold run of this version produced a corrupted result
    (rel 3.3) that never reproduced: 20/20 stress runs + all normal
    executions are exact.  Suspect NTFF-capture timing perturbation;
    re-verify with /tmp/stress.py-style loops if touching DMA deps.
"""

import numpy as np

N = 4096          # points per cloud
P = 128           # partitions
T = N // P        # 32 row tiles
K = 16            # neighbors
CHUNK = 1024      # top-k chunk width (2 psum banks)
NCHUNK = N // CHUNK
DIAG_NEG = -60000.0  # added on the diagonal (self distance); fits fp16
FILL_NEG = -3.0e38   # match_replace fill
SQ = 256.0           # d2 quantization scale for the merge keys
BRND = 3.0 * 2.0**22  # fp32 round-to-int bias
B4096 = BRND * 4096.0

_PROG_CACHE = {}


def _build_program(stage="full", reps=1):
    """Build + compile the single-core Bass program (shared by all 8 cores).

    stage: "mm" | "topk" | "idx" | "gather" | "full" — debug prefixes of
    the pipeline; anything but "full" writes intermediate checksums instead.
    reps: repeat the whole pipeline (timing harness: marginal wall per rep).
    """
    import concourse.bacc as bacc
    import concourse.bass as bass
    import concourse.mybir as mybir
    import concourse.tile as tile
    from concourse.tile_rust import add_dep_helper

    dt = mybir.dt
    AF = mybir.ActivationFunctionType
    Alu = mybir.AluOpType

    nc = bacc.Bacc("TRN2", target_bir_lowering=False, debug=False)

    lhsT6 = nc.dram_tensor("lhsT6", [6, N], dt.float16, kind="ExternalInput")
    rhs6 = nc.dram_tensor("rhs6", [6, N], dt.float16, kind="ExternalInput")
    xyz = nc.dram_tensor("xyz", [N, 3], dt.float32, kind="ExternalInput")
    # fp16 copy of the coordinates, padded to 4 components, for the
    # neighbor-fetch table: ap_gather cost is ~11ns per 4B word per idx,
    # so d=4 fp16 (8B) beats d=3 fp32 (12B) by ~22%% on the Q7 floor.
    # Only neighbor coords quantize; own-point coords stay exact fp32.
    xyz16 = nc.dram_tensor("xyz16", [N, 4], dt.float16, kind="ExternalInput")
    nrm = nc.dram_tensor("nrm", [N, 3], dt.float32, kind="ExternalInput")
    eye = nc.dram_tensor("eye", [P, P], dt.float16, kind="ExternalInput")
    # DIAG_NEG*I at columns 384:512 of a zero [P, 896]; slicing
    # [384-off : 896-off] yields a [P, 512] half-chunk row with the negative
    # diagonal at columns off:off+P
    negpad = nc.dram_tensor("negpad", [P, 896], dt.float16, kind="ExternalInput")
    # per-column chunk offset (c//8)*1024 to globalize chunk-local indices
    choff = nc.dram_tensor("choff", [P, 32], dt.float32, kind="ExternalInput")
    # quadrant mask for the wrapped-list build: 1 where p-half XOR f-half
    qmask = nc.dram_tensor("qmask", [P, 64], dt.float32, kind="ExternalInput")
    kap_d = nc.dram_tensor("kappa", [N, 1], dt.float32, kind="ExternalOutput")
    std_d = nc.dram_tensor("std", [N, 1], dt.float32, kind="ExternalOutput")

    def bcast_mid(ap, k):
        # [P, (1,) c] -> [P, k, c] with a stride-0 middle dim
        return bass.AP(ap.tensor, ap.offset, [ap.ap[0], [0, k], ap.ap[-1]])

    def dram_bcast(t_handle, n_elem):
        # DRAM tensor broadcast to all 128 partitions: [[0,128],[1,n]]
        ap = t_handle.ap()
        return bass.AP(ap.tensor, 0, [[0, P], [1, n_elem]])

    with tile.TileContext(nc) as tc:
        with (
            tc.tile_pool(name="const", bufs=1) as constp,
            tc.tile_pool(name="srow", bufs=4) as sp,
            tc.tile_pool(name="psum", bufs=4, space="PSUM") as pp,
            tc.tile_pool(name="small", bufs=4) as smp,
            tc.tile_pool(name="call", bufs=2) as callp,
            tc.tile_pool(name="rep", bufs=2) as repp,
            tc.tile_pool(name="idxp", bufs=1) as idxp,
        ):
            lh = constp.tile_from(lhsT6.ap())
            rh = constp.tile_from(rhs6.ap())
            ey = constp.tile_from(eye.ap())
            npd = constp.tile_from(negpad.ap())
            co = constp.tile_from(choff.ap())
            qm = constp.tile_from(qmask.ap())
            # replicated coordinate table: every partition holds all of
            # xyz16 (fp16, 4 components per point)
            xyztab = constp.tile([P, N * 4], dt.float16)
            nc.sync.dma_start(xyztab[:], dram_bcast(xyz16, N * 4))
            # wrapped int16 index lists for all 16 ap_gather calls
            L_all = idxp.tile([P, (T // 4) * 64], dt.int16)
            kaptab = idxp.tile([P, N], dt.float32)
            # all tiles' own coords/normals in one DMA: [p, t, c] <- row t*P+p
            xi_all = constp.tile([P, T, 3], dt.float32)
            nc.sync.dma_start(
                xi_all[:], xyz.ap().rearrange("(t p) c -> p t c", p=P)
            )
            ni_all = constp.tile([P, T, 3], dt.float32)
            nc.sync.dma_start(
                ni_all[:], nrm.ap().rearrange("(t p) c -> p t c", p=P)
            )

            # call schedule: 7 calls of 4 tiles, then 2 calls of 2 tiles —
            # the smaller final gathers shorten the post-scan tail.
            call_plan = ([(4 * c, 4) for c in range(6)]
                         + [(24, 2), (26, 2), (28, 2), (30, 2)])
            call_w0 = {cs: w for cs, w in call_plan}
            for _rep in range(reps):
                # ---------------- phase A: knn + kappa ----------------
                jf_call = None
                cur_cs, cur_w, cur_col = 0, 4, 0
                kap_stores = []
                for t in range(T):
                    cand = smp.tile([P, 32], dt.float32, tag="cand")
                    candi = smp.tile([P, 32], dt.uint32, tag="candi")
                    if t in call_w0:
                        if jf_call is not None:
                            cur_col += 16 * cur_w
                        cur_cs, cur_w = t, call_w0[t]
                        jf_call = callp.tile(
                            [P, 16 * cur_w], dt.float32, tag=f"jf{cur_w}",
                            name=f"jf{cur_w}_{t}",
                        )
                    cd, off = (t * P) // CHUNK, (t * P) % CHUNK
                    for c in range(NCHUNK):
                        ps = pp.tile([P, CHUNK], dt.float32, tag="ps")
                        for h in range(2):
                            b = 2 * c + h
                            hs = ps[:, h * 512 : (h + 1) * 512]
                            diag_here = c == cd and (off // 512) == h
                            hoff = off % 512
                            nc.tensor.matmul(
                                out=hs,
                                lhsT=lh[:, t * P : (t + 1) * P],
                                rhs=rh[:, b * 512 : (b + 1) * 512],
                                start=True,
                                stop=not diag_here,
                            )
                            if diag_here:
                                nc.tensor.matmul(
                                    out=hs,
                                    lhsT=ey[:],
                                    rhs=npd[:, 384 - hoff : 896 - hoff],
                                    start=False,
                                    stop=True,
                                )
                        Sbt = sp.tile([P, CHUNK], dt.float32, tag="Sb")
                        Sb = Sbt[:]
                        nc.scalar.copy(Sb, ps[:])
                        # per-chunk top-8 values + chunk-local indices
                        nc.vector.max(cand[:, c * 8 : c * 8 + 8], Sb)
                        nc.vector.max_index(
                            candi[:, c * 8 : c * 8 + 8],
                            cand[:, c * 8 : c * 8 + 8],
                            Sb,
                        )

                    if stage == "mm":
                        # per-chunk top-8 already captures the row max
                        chk = smp.tile([P, 1], dt.float32, tag="chk")
                        nc.vector.tensor_reduce(
                            chk[:], cand[:], axis=mybir.AxisListType.X, op=Alu.max
                        )
                        nc.sync.dma_start(std_d.ap()[t * P : (t + 1) * P, :], chk[:])
                        continue

                    # merge by packed keys: key = 4096*(2047 + round(v*256)) + j
                    candif = smp.tile([P, 32], dt.float32, tag="candif")
                    nc.scalar.copy(candif[:], candi[:])
                    candg = smp.tile([P, 32], dt.float32, tag="candg")
                    nc.vector.tensor_tensor(
                        out=candg[:], in0=candif[:], in1=co[:], op=Alu.add
                    )
                    q = smp.tile([P, 32], dt.float32, tag="q")
                    nc.scalar.activation(
                        q[:], cand[:], AF.Copy, bias=2047.0 + BRND, scale=SQ
                    )
                    t1 = smp.tile([P, 32], dt.float32, tag="t1")
                    nc.scalar.activation(
                        t1[:], q[:], AF.Copy, bias=-B4096, scale=4096.0
                    )
                    key = smp.tile([P, 32], dt.float32, tag="key")
                    nc.vector.tensor_tensor(
                        out=key[:], in0=t1[:], in1=candg[:], op=Alu.add
                    )
                    k16 = smp.tile([P, 16], dt.float32, tag="k16")
                    key_mr = smp.tile([P, 32], dt.float32, tag="key_mr")
                    nc.vector.max(k16[:, 0:8], key[:])
                    nc.vector.match_replace(key_mr[:], k16[:, 0:8], key[:], FILL_NEG)
                    nc.vector.max(k16[:, 8:16], key_mr[:])

                    if stage == "topk":
                        chk = smp.tile([P, 1], dt.float32, tag="chk")
                        nc.vector.tensor_reduce(
                            chk[:], k16[:], axis=mybir.AxisListType.X, op=Alu.add
                        )
                        nc.sync.dma_start(std_d.ap()[t * P : (t + 1) * P, :], chk[:])
                        continue

                    # unpack j = k16 mod 4096 (exact fp32 integer arithmetic)
                    u1 = smp.tile([P, 16], dt.float32, tag="u1")
                    nc.scalar.activation(
                        u1[:], k16[:], AF.Copy,
                        bias=-0.4998779296875, scale=1.0 / 4096.0,
                    )
                    u2a = smp.tile([P, 16], dt.float32, tag="u2a")
                    nc.scalar.activation(u2a[:], u1[:], AF.Copy, bias=BRND)
                    u2 = smp.tile([P, 16], dt.float32, tag="u2")
                    nc.scalar.activation(u2[:], u2a[:], AF.Copy, bias=-BRND)
                    tl = t - cur_cs
                    jf = jf_call[:, tl * 16 : tl * 16 + 16]
                    nc.vector.scalar_tensor_tensor(
                        out=jf, in0=u2[:], scalar=-4096.0, in1=k16[:],
                        op0=Alu.mult, op1=Alu.add,
                    )

                    if stage == "idx":
                        chk = smp.tile([P, 1], dt.float32, tag="chk")
                        nc.vector.tensor_reduce(
                            chk[:], jf, axis=mybir.AxisListType.X, op=Alu.add
                        )
                        nc.sync.dma_start(std_d.ap()[t * P : (t + 1) * P, :], chk[:])
                        continue

                    if t != cur_cs + cur_w - 1:
                        continue

                    # ---- per W-tile call: list build + gather + kappa ----
                    W = cur_w
                    WC = 16 * W  # list columns
                    # wrapped list Lf[32A+ah*16+k, mh*16+b] =
                    #   jf_call[32A+ah*16+b, mh*16+k] = o32[32A+mh*16+k, ah*16+b]
                    # via 32x32 stream transpose + 16-lane shuffle + quad copies
                    o32 = smp.tile([P, WC], dt.float32, tag=f"o32w{W}",
                                   name=f"o32w{W}_{t}")
                    nc.vector.transpose(o32[:], jf_call[:])
                    sh = smp.tile([P, WC], dt.float32, tag=f"shw{W}",
                                  name=f"shw{W}_{t}")
                    nc.vector.stream_shuffle(
                        sh[:], o32[:], mask=[(i + 16) % 32 for i in range(32)]
                    )
                    # diag quads from o32; off-diag quads (partition-half XOR
                    # free-half) overwritten from the 16-lane-shifted copy,
                    # whose free-halves swap: use a free-half-swapped view of
                    # sh so one predicated copy works
                    Lf = smp.tile([P, WC], dt.float32, tag=f"Lfw{W}",
                                  name=f"Lfw{W}_{t}")
                    nc.scalar.copy(Lf[:], o32[:])
                    # Lf = o32 + qm * (sh_freehalfswap - o32)   (exact: ints)
                    sh_ap = sh[:]
                    sh_sw = bass.AP(
                        sh_ap.tensor, sh_ap.offset + 16,
                        [sh_ap.ap[0], [32, WC // 32], [-16, 2], [1, 16]],
                    )
                    dq = smp.tile([P, WC], dt.float32, tag=f"dqw{W}",
                                  name=f"dqw{W}_{t}")
                    dq3 = dq[:].rearrange("p (f h b) -> p f h b", h=2, b=16)
                    nc.vector.tensor_tensor(
                        out=dq3, in0=sh_sw,
                        in1=Lf[:].rearrange("p (f h b) -> p f h b", h=2, b=16),
                        op=Alu.subtract,
                    )
                    nc.vector.tensor_tensor(
                        out=dq[:], in0=dq[:], in1=qm[:, 0:WC], op=Alu.mult
                    )
                    nc.vector.tensor_tensor(
                        out=Lf[:], in0=Lf[:], in1=dq[:], op=Alu.add
                    )
                    L16 = L_all[:, cur_col : cur_col + WC]
                    nc.scalar.copy(L16, Lf[:])

                    # one d=3 ap_gather: rep3[p in core a, (mh*16+b)*16+k] =
                    #   xyz[idx of point((cs+mh)*128 + a*16+b, k)]
                    rep3 = callp.tile([P, W * 256 * 4], dt.float16,
                                      tag=f"rep3w{W}", name=f"rep3w{W}_{t}")
                    nc.gpsimd.ap_gather(
                        out_ap=rep3[:],
                        in_ap=xyztab[:],
                        idxs_ap=L16,
                        channels=P,
                        num_elems=N,
                        d=4,
                        num_idxs=W * 256,
                    )
                    # compact to per-point layout: nnc16[a*16+b, mh*64+kz] =
                    #   rep3[a*16, mh*1024 + b*64 + kz]   (64 fp16 per point)
                    nnc16 = callp.tile([P, W * K * 4], dt.float16,
                                       tag=f"nnc16w{W}", name=f"nnc16w{W}_{t}")
                    r3ap = rep3[:]
                    for mh in range(W):
                        src = bass.AP(
                            r3ap.tensor,
                            r3ap.offset + mh * 1024,
                            [[r3ap.ap[0][0] * 16, 8], [64, 16], [1, 64]],
                        )
                        nc.sync.dma_start(
                            nnc16[:, mh * 64 : (mh + 1) * 64], src
                        )
                    nnc = callp.tile([P, W * K * 4], dt.float32,
                                     tag=f"nncw{W}", name=f"nncw{W}_{t}")
                    nc.scalar.copy(nnc[:], nnc16[:])

                    if stage == "gather":
                        for mh in range(W):
                            tt = cur_cs + mh
                            chk = smp.tile([P, 1], dt.float32, tag="chk")
                            nc.vector.tensor_reduce(
                                chk[:], nnc[:, mh * 64 : (mh + 1) * 64],
                                axis=mybir.AxisListType.X, op=Alu.add,
                            )
                            nc.sync.dma_start(
                                std_d.ap()[tt * P : (tt + 1) * P, :], chk[:]
                            )
                        continue

                    # kappa for all W tiles of the call in one wide chain
                    # (nnc holds 4 components per neighbor; view picks 0:3)
                    nnc_ap = nnc[:]
                    nn4 = bass.AP(
                        nnc_ap.tensor, nnc_ap.offset,
                        [nnc_ap.ap[0], [64, W], [4, K], [1, 3]],
                    )
                    xi4 = xi_all[:, cur_cs : cur_cs + W, :]
                    ni4 = ni_all[:, cur_cs : cur_cs + W, :]

                    def bcast_k4(ap):
                        # [P, W, 3] -> [P, W, K, 3] with a stride-0 K dim
                        return bass.AP(
                            ap.tensor, ap.offset,
                            [ap.ap[0], ap.ap[1], [0, K], ap.ap[2]],
                        )

                    v = smp.tile([P, W * K * 3], dt.float32, tag=f"vw{W}",
                                 name=f"vw{W}_{t}")
                    v4 = v[:].rearrange("p (t k c) -> p t k c", k=K, c=3)
                    nc.vector.tensor_tensor(
                        out=v4, in0=nn4, in1=bcast_k4(xi4), op=Alu.subtract
                    )
                    vn = smp.tile([P, W * K * 3], dt.float32, tag=f"vnw{W}",
                                  name=f"vnw{W}_{t}")
                    vn4 = vn[:].rearrange("p (t k c) -> p t k c", k=K, c=3)
                    nc.vector.tensor_tensor(
                        out=vn4, in0=v4, in1=bcast_k4(ni4), op=Alu.mult
                    )
                    dot = smp.tile([P, W * K], dt.float32, tag=f"dotw{W}",
                                   name=f"dotw{W}_{t}")
                    nc.vector.tensor_reduce(
                        dot[:].rearrange("p (t k) -> p t k", k=K), vn4,
                        axis=mybir.AxisListType.X, op=Alu.add,
                    )
                    v2 = smp.tile([P, W * K * 3], dt.float32, tag=f"v2w{W}",
                                  name=f"v2w{W}_{t}")
                    v24 = v2[:].rearrange("p (t k c) -> p t k c", k=K, c=3)
                    nc.vector.tensor_tensor(out=v24, in0=v4, in1=v4, op=Alu.mult)
                    n2 = smp.tile([P, W * K], dt.float32, tag=f"n2w{W}",
                                  name=f"n2w{W}_{t}")
                    nc.vector.tensor_reduce(
                        n2[:].rearrange("p (t k) -> p t k", k=K), v24,
                        axis=mybir.AxisListType.X, op=Alu.add,
                    )
                    # clip ||v||^2 at 1e-24 (reference clips ||v|| at 1e-12)
                    nc.vector.tensor_scalar_max(n2[:], n2[:], 1e-24)
                    ri = smp.tile([P, W * K], dt.float32, tag=f"riw{W}",
                                  name=f"riw{W}_{t}")
                    nc.vector.reciprocal(ri[:], n2[:])
                    rs = smp.tile([P, W * K], dt.float32, tag=f"rsw{W}",
                                  name=f"rsw{W}_{t}")
                    nc.scalar.activation(rs[:], ri[:], AF.Sqrt)
                    sc = smp.tile([P, W * K], dt.float32, tag=f"scw{W}",
                                  name=f"scw{W}_{t}")
                    nc.vector.tensor_tensor(
                        out=sc[:], in0=dot[:], in1=rs[:], op=Alu.mult
                    )
                    kap = smp.tile([P, W], dt.float32, tag=f"kapw{W}",
                                   name=f"kapw{W}_{t}")
                    nc.vector.tensor_reduce(
                        kap[:].rearrange("p (t o) -> p t o", o=1),
                        sc[:].rearrange("p (t k) -> p t k", k=K),
                        axis=mybir.AxisListType.X,
                        op=Alu.add,
                        apply_absolute_value=True,
                    )  # = 16 * kappa
                    for mh in range(W):
                        tt = cur_cs + mh
                        kst = nc.sync.dma_start(
                            kap_d.ap()[tt * P : (tt + 1) * P, :],
                            kap[:, mh : mh + 1],
                        )
                        kap_stores.append(kst)

                # ---------------- phase B: neighbor-kappa std ----------------
                ncall = 4 if stage == "full" else 0
                if ncall:
                    # kaptab broadcast in 8 chunks, each only waiting on its
                    # own 4 tiles' kappa stores -> overlaps phase A
                    kap_ap = kap_d.ap()
                    for qc in range(8):
                        seg = bass.AP(
                            kap_ap.tensor, qc * 512, [[0, P], [1, 512]]
                        )
                        kb = nc.sync.dma_start(
                            kaptab[:, qc * 512 : (qc + 1) * 512], seg
                        )
                        for kst in kap_stores[qc * 4 : (qc + 1) * 4]:
                            add_dep_helper(kb.ins, kst.ins, True,
                                           "kaptab chunk after kappa stores")
                for call in range(ncall):
                    rep1 = repp.tile([P, 2048], dt.float32, tag="rep1")
                    nc.gpsimd.ap_gather(
                        out_ap=rep1[:],
                        in_ap=kaptab[:],
                        idxs_ap=L_all[:, call * 128 : (call + 1) * 128],
                        channels=P,
                        num_elems=N,
                        d=1,
                        num_idxs=2048,
                    )
                    nnk = smp.tile([P, 8 * K], dt.float32, tag="nnk")
                    r1ap = rep1[:]
                    for mh in range(8):
                        src = bass.AP(
                            r1ap.tensor,
                            r1ap.offset + mh * 256,
                            [[r1ap.ap[0][0] * 16, 8], [16, 16], [1, 16]],
                        )
                        eng = nc.sync if mh % 2 == 0 else nc.scalar
                        eng.dma_start(nnk[:, mh * 16 : (mh + 1) * 16], src)
                    # wide std math for all 8 tiles of the call:
                    # ss = sum k^2 - 16*mean^2  (safe: ~6e-6 rel cancellation)
                    nnk3 = nnk[:].rearrange("p (t k) -> p t k", k=K)
                    sm = smp.tile([P, 8], dt.float32, tag="sm")
                    nc.vector.tensor_reduce(
                        sm[:], nnk3, axis=mybir.AxisListType.X, op=Alu.add
                    )
                    mn = smp.tile([P, 8], dt.float32, tag="mn")
                    nc.vector.tensor_scalar_mul(mn[:], sm[:], 1.0 / K)
                    sq = smp.tile([P, 8 * K], dt.float32, tag="sq")
                    nc.vector.tensor_tensor(
                        out=sq[:], in0=nnk[:], in1=nnk[:], op=Alu.mult
                    )
                    r2 = smp.tile([P, 8], dt.float32, tag="r2")
                    nc.vector.tensor_reduce(
                        r2[:], sq[:].rearrange("p (t k) -> p t k", k=K),
                        axis=mybir.AxisListType.X, op=Alu.add,
                    )
                    m2 = smp.tile([P, 8], dt.float32, tag="m2")
                    nc.vector.tensor_tensor(
                        out=m2[:], in0=mn[:], in1=mn[:], op=Alu.mult
                    )
                    ss = smp.tile([P, 8], dt.float32, tag="ss")
                    nc.vector.scalar_tensor_tensor(
                        out=ss[:], in0=m2[:], scalar=-float(K), op0=Alu.mult,
                        in1=r2[:], op1=Alu.add,
                    )
                    stdt = smp.tile([P, 8], dt.float32, tag="stdt")
                    # std = sqrt(ss/(K-1))/K  (kappa was stored scaled by K)
                    nc.scalar.activation(
                        stdt[:], ss[:], AF.Sqrt, scale=1.0 / ((K - 1) * K * K)
                    )
                    for mh in range(8):
                        tt = 8 * call + mh
                        nc.sync.dma_start(
                            std_d.ap()[tt * P : (tt + 1) * P, :],
                            stdt[:, mh : mh + 1],
                        )

    nc.compile()
    return nc


def get_program():
    if "nc" not in _PROG_CACHE:
        _PROG_CACHE["nc"] = _build_program()
    return _PROG_CACHE["nc"]


def make_in_map(x3n: np.ndarray, nrm3n: np.ndarray) -> dict:
    """Per-core inputs. x3n, nrm3n: (3, N) float32."""
    x = np.ascontiguousarray(x3n, dtype=np.float32)          # (3, N)
    xyz = np.ascontiguousarray(x.T)                          # (N, 3)
    nrm = np.ascontiguousarray(np.asarray(nrm3n, np.float32).T)
    sq = (x * x).sum(axis=0, dtype=np.float32)               # (N,)
    sq_hi = sq.astype(np.float16).astype(np.float32)
    sq_lo = sq - sq_hi
    ones = np.ones((N,), np.float32)
    rhs6 = np.ascontiguousarray(
        np.stack([x[0], x[1], x[2], ones, sq_hi, sq_lo]).astype(np.float16)
    )
    lhsT6 = np.ascontiguousarray(
        np.stack([2 * x[0], 2 * x[1], 2 * x[2], -sq, -ones, -ones]).astype(
            np.float16
        )
    )
    eye = np.eye(P, dtype=np.float16)
    negpad = np.zeros((P, 896), np.float16)
    negpad[:, 384:512] = np.float16(DIAG_NEG) * np.eye(P, dtype=np.float16)
    choff = np.broadcast_to(
        ((np.arange(32) // 8) * CHUNK).astype(np.float32), (P, 32)
    ).copy()
    pq = (np.arange(P)[:, None] % 32 >= 16)
    fq = (np.arange(64)[None, :] % 32 >= 16)
    qmask = (pq ^ fq).astype(np.float32)
    xyz16 = np.zeros((N, 4), np.float16)
    xyz16[:, 0:3] = xyz.astype(np.float16)
    return {
        "lhsT6": lhsT6,
        "rhs6": rhs6,
        "xyz": xyz,
        "xyz16": xyz16,
        "nrm": nrm,
        "eye": eye,
        "negpad": negpad,
        "choff": choff,
        "qmask": qmask,
    }


def combine(std_vecs: list) -> np.ndarray:
    """std_vecs: 8 arrays (N,) — cores 0-3 ori batches, 4-7 adv batches."""
    dists = []
    for b in range(4):
        diff = (
            std_vecs[b].astype(np.float64)
            - std_vecs[4 + b].astype(np.float64)
            + 1e-6
        )
        dists.append(np.sqrt((diff * diff).sum()))
    return np.asarray(np.mean(dists), dtype=np.float32)


def kernel(ori_data, adv_data, ori_normal):
    from concourse.bass_utils import run_bass_kernel_spmd

    nc = get_program()
    in_maps = []
    for cloud in (ori_data, adv_data):
        for b in range(4):
            in_maps.append(make_in_map(cloud[b], ori_normal[b]))
    res = run_bass_kernel_spmd(nc, in_maps, core_ids=list(range(8)))
    std_vecs = [r["std"][:, 0] for r in res.results]
    return combine(std_vecs)
